# revision 1
# baseline (speedup 1.0000x reference)
"""Trainium2 Bass kernel for DeepGCNLayer(GENConv softmax-aggr) + encoder.

Computation (see reference):
  h  = relu(batchnorm(x))                       # BN0 over all N nodes
  msg_e = relu(h[src_e]) + eps = h[src_e] + eps # h >= 0 already
  agg_v = softmax-weighted mean of msg over incoming edges (t=1)
  z0 = agg + h
  z1 = relu(BN1(z0 @ W1 + b1)); z2 = relu(BN2(z1 @ W2 + b2))
  out = (x + z2 @ W3 + b3) @ We + be

Strategy (8 NeuronCores, SPMD single program):
  * Host packs nodes into 128-slot tiles balanced by in-degree, tiles
    assigned to cores; per-core activations live transposed [ch, nodes].
  * BN stats computed on-device, AllReduce'd ([128,2]/[128,4] f32).
  * h computed per-shard, AllGather'd into a replicated row table.
  * Edge phase: per 128-edge block, HW-DGE indirect gather of h rows,
    exp on ACT, one-hot selection matrix (DVE compare vs iota), PE
    matmul S^T @ [e | m*e] accumulated in PSUM per node tile.
  * MLP phase: weight-stationary PE matmuls over 512-col node blocks,
    BN via ACT accum_out stats + per-partition scale/bias activation.
  * Empty node slots contribute a constant column; corrected in BN
    stats using the reserved always-empty last slot times n_empty.
"""

import math
import numpy as np
from contextlib import ExitStack

import concourse.bass as bass
import concourse.tile as tile
from concourse import bacc, mybir
from concourse.bass_utils import run_bass_kernel_spmd
from concourse.masks import make_identity

F32 = mybir.dt.float32
F32R = mybir.dt.float32r
BF16 = mybir.dt.bfloat16
I32 = mybir.dt.int32
AF = mybir.ActivationFunctionType
OP = mybir.AluOpType

NCORES = 8
EPS_GEN = 1e-7
EPS_BN = 1e-5
DEN_EPS = 1e-16

# full-size problem config (hardcoded; harness calls kernel() directly)
N_FULL = 50000
C = 128
H = 256


# --------------------------------------------------------------------------
# host-side graph partitioning
# --------------------------------------------------------------------------

def _plan(n_nodes, src, dst):
    """Pack nodes into 128-slot bins balanced by in-degree; assign bins to
    cores; build per-core edge blocks and permutation arrays."""
    import heapq

    tiles = math.ceil(n_nodes / (NCORES * 128) + 0.02)  # slack for reserve
    if tiles * NCORES * 128 - NCORES < n_nodes:
        tiles += 1
    nbins = NCORES * tiles
    nloc = tiles * 128

    deg = np.bincount(dst, minlength=n_nodes).astype(np.int64)
    order = np.argsort(-deg, kind="stable")
    heap = [(0, i) for i in range(nbins)]
    heapq.heapify(heap)
    bin_nodes = [[] for _ in range(nbins)]
    bin_load = np.zeros(nbins, np.int64)
    for g in order:
        d = int(deg[g])
        spill = []
        while True:
            load, b = heapq.heappop(heap)
            if len(bin_nodes[b]) < 128:
                bin_nodes[b].append(g)
                bin_load[b] = load + d
                heapq.heappush(heap, (load + d, b))
                break
            spill.append((load, b))
        for it in spill:
            heapq.heappush(heap, it)

    # snake-assign bins to cores by load
    border = np.argsort(-bin_load, kind="stable")
    core_bins = [[] for _ in range(NCORES)]
    for i, b in enumerate(border):
        r = i // NCORES
        c = i % NCORES if r % 2 == 0 else NCORES - 1 - (i % NCORES)
        core_bins[c].append(b)
    # per-core: order bins by load desc; reserved empty slot = last slot of
    # last tile -> last bin must have <= 127 nodes
    for c in range(NCORES):
        core_bins[c].sort(key=lambda b: -bin_load[b])
        last = core_bins[c][-1]
        if len(bin_nodes[last]) >= 128:
            g = bin_nodes[last].pop()  # move lowest-degree node
            moved = False
            for b in core_bins[c]:
                if b != last and len(bin_nodes[b]) < 128:
                    bin_nodes[b].append(g)
                    moved = True
                    break
            if not moved:
                for c2 in range(NCORES):
                    for b in core_bins[c2]:
                        if b != last and len(bin_nodes[b]) < 128:
                            bin_nodes[b].append(g)
                            moved = True
                            break
                    if moved:
                        break
            assert moved

    core_of = np.empty(n_nodes, np.int64)
    tile_of = np.empty(n_nodes, np.int64)
    slot_of = np.empty(n_nodes, np.int64)
    for c in range(NCORES):
        for k, b in enumerate(core_bins[c]):
            for s, g in enumerate(bin_nodes[b]):
                core_of[g] = c
                tile_of[g] = k
                slot_of[g] = s
    loc_of = tile_of * 128 + slot_of
    prow = core_of * nloc + loc_of

    # per (core, tile) edge loads -> uniform-across-core block counts
    ecore = core_of[dst]
    ekey = ecore * tiles + tile_of[dst]
    counts = np.bincount(ekey, minlength=nbins)
    loads = counts.reshape(NCORES, tiles)
    B = np.maximum(1, (loads.max(axis=0) + 127) // 128).astype(np.int64)
    colofs = np.concatenate([[0], np.cumsum(B)])
    NB = int(colofs[-1])

    # build edge arrays
    esrc = np.zeros((NCORES, 128, NB), np.int32)
    edst = np.full((NCORES, 128, NB), 255.0, np.float32)
    eorder = np.argsort(ekey, kind="stable")
    sk = ekey[eorder]
    starts = np.concatenate([[0], np.cumsum(np.bincount(sk, minlength=nbins))])
    pos = np.arange(len(sk)) - starts[sk]
    lane = pos % 128
    blk = pos // 128
    ec = sk // tiles
    ek = sk % tiles
    col = colofs[ek] + blk
    esrc[ec, lane, col] = prow[src[eorder]].astype(np.int32)
    edst[ec, lane, col] = slot_of[dst[eorder]].astype(np.float32)

    n_empty = np.array(
        [nloc - sum(len(bin_nodes[b]) for b in core_bins[c]) for c in range(NCORES)],
        np.float32,
    )
    return dict(
        tiles=tiles, nloc=nloc, ntab=NCORES * nloc, NB=NB,
        B=B, colofs=colofs, core_of=core_of, loc_of=loc_of, prow=prow,
        esrc=esrc, edst=edst, n_empty=n_empty,
    )


def _node_blocks(nloc):
    out = []
    o = 0
    while o < nloc:
        w = min(512, nloc - o)
        out.append((o, w))
        o += w
    return out


# --------------------------------------------------------------------------
# kernel build
# --------------------------------------------------------------------------

def _indirect_on(eng, queue_name, out, in_, idx_ap):
    """Emit an indirect row-gather InstDMACopy on a chosen engine/queue."""
    from concourse.bass import BassSymbolicTensorAccessPattern
    assert in_.space == bass.MemorySpace.DRAM
    out_ap = eng.lower_ap_dma(out, for_indirect_dma=True)
    in_ap = eng.lower_ap_dma(in_, for_indirect_dma=True)
    offset_ap = eng.lower_ap_dma(idx_ap)
    assert len(in_ap) == 1 and len(out_ap) == 1 and len(offset_ap) == 1
    in_ap.append(offset_ap[0])
    coef = in_.shape[1]
    in_ap[0].dynamic_ap_info = mybir.DynamicAccessPatternInfo(
        c=0, actual_ap=out.ap,
        indirect_dim_max_index=in_.shape[0],
        offset_expr=[mybir.DynamicAccessPatternOffsetExpr(
            coef=coef,
            aff_expr=mybir.DynamicAccessPatternOffsetExprAffExpr(
                kind="IndirectArgId", arg_id=1))],
    )
    return eng.add_instruction(mybir.InstDMACopy(
        name=eng.bass.get_next_instruction_name(),
        queue=queue_name, mode="Copy", ins=in_ap, outs=out_ap,
        oob_is_err=True, cce_op=OP.bypass))


def _reg_const(nc, val, dtype=F32):
    t = nc.alloc_sbuf_tensor(f"constap-{val}", [128, 1], dtype)
    nc.gpsimd.memset(t.ap(), val)
    nc.const_aps.aps[(dtype, val)] = t.ap()


def _build(plan, n_real):
    tiles, nloc, ntab, NB = plan["tiles"], plan["nloc"], plan["ntab"], plan["NB"]
    B, colofs = plan["B"], plan["colofs"]
    blocks = _node_blocks(nloc)
    nblk = len(blocks)
    inv_n = 1.0 / float(n_real)

    nc = bacc.Bacc("TRN2", target_bir_lowering=False, debug=False,
                   num_devices=NCORES)
    for v in (EPS_GEN, EPS_BN, DEN_EPS):
        _reg_const(nc, v)
    nc.all_engine_barrier()

    xT_d = nc.declare_dram_parameter("xT", [128, nloc], F32, isOutput=False)
    esrc_d = nc.declare_dram_parameter("esrc", [128, NB], I32, isOutput=False)
    edst_d = nc.declare_dram_parameter("edst", [128, NB], F32, isOutput=False)
    iota_d = nc.declare_dram_parameter("iota", [128, 128], F32, isOutput=False)
    w1_d = nc.declare_dram_parameter("w1", [128, H], F32R, isOutput=False)
    w2_d = nc.declare_dram_parameter("w2", [128, 2 * H], F32R, isOutput=False)
    w3_d = nc.declare_dram_parameter("w3", [128, H], F32R, isOutput=False)
    we_d = nc.declare_dram_parameter("we", [128, C], F32R, isOutput=False)
    pc_d = nc.declare_dram_parameter("pcols", [128, 17], F32, isOutput=False)
    out_d = nc.declare_dram_parameter("outT", [128, nloc], F32, isOutput=True)

    with ExitStack() as ctx:
        tc = ctx.enter_context(tile.TileContext(nc))
        cst = ctx.enter_context(tc.tile_pool(name="cst", bufs=1))
        big = ctx.enter_context(tc.tile_pool(name="big", bufs=1))
        wk = ctx.enter_context(tc.tile_pool(name="wk", bufs=4))
        ps = ctx.enter_context(tc.tile_pool(name="ps", bufs=2, space="PSUM"))
        dr = ctx.enter_context(tc.tile_pool(name="dr", bufs=1, space="DRAM"))

        # ---- load constants / params
        xT = big.tile([128, nloc], F32, name="xT", tag="slot_xT")
        nc.sync.dma_start(xT[:], xT_d[:])
        esrc = cst.tile([128, NB], I32, name="esrc")
        nc.sync.dma_start(esrc[:], esrc_d[:])
        edst = cst.tile([128, NB], F32, name="edst")
        nc.sync.dma_start(edst[:], edst_d[:])
        iota = cst.tile([128, 128], F32, name="iota")
        nc.sync.dma_start(iota[:], iota_d[:])
        w1 = cst.tile([128, H], F32R, name="w1")
        nc.sync.dma_start(w1[:], w1_d[:])
        w2 = cst.tile([128, 2 * H], F32R, name="w2")
        nc.sync.dma_start(w2[:], w2_d[:])
        w3 = cst.tile([128, H], F32R, name="w3")
        nc.sync.dma_start(w3[:], w3_d[:])
        we = cst.tile([128, C], F32R, name="we")
        nc.sync.dma_start(we[:], we_d[:])
        pc = cst.tile([128, 17], F32, name="pc")
        nc.sync.dma_start(pc[:], pc_d[:])
        ident = cst.tile([128, 128], F32, name="ident")
        make_identity(nc, ident[:])

        g0, b0 = pc[:, 0:1], pc[:, 1:2]
        b1 = [pc[:, 2:3], pc[:, 3:4]]
        g1 = [pc[:, 4:5], pc[:, 5:6]]
        be1 = [pc[:, 6:7], pc[:, 7:8]]
        b2 = [pc[:, 8:9], pc[:, 9:10]]
        g2 = [pc[:, 10:11], pc[:, 11:12]]
        be2 = [pc[:, 12:13], pc[:, 13:14]]
        b3, be_enc, n_emp = pc[:, 14:15], pc[:, 15:16], pc[:, 16:17]

        # ---- helper: BN params from allreduced [sum, sumsq] cols
        def bn_params(st_sum, st_ssq, g_ap, beta_ap, name):
            mean = cst.tile([128, 1], F32, name=f"{name}_mean")
            nc.vector.tensor_scalar(out=mean[:], in0=st_sum, scalar1=inv_n,
                                    scalar2=None, op0=OP.mult)
            msq = cst.tile([128, 1], F32, name=f"{name}_msq")
            nc.vector.tensor_scalar(out=msq[:], in0=st_ssq, scalar1=inv_n,
                                    scalar2=None, op0=OP.mult)
            var = cst.tile([128, 1], F32, name=f"{name}_var")
            nc.vector.tensor_tensor(out=var[:], in0=mean[:], in1=mean[:],
                                    op=OP.mult)
            nc.vector.tensor_tensor(out=var[:], in0=msq[:], in1=var[:],
                                    op=OP.subtract)
            sd = cst.tile([128, 1], F32, name=f"{name}_sd")
            nc.scalar.activation(sd[:], var[:], AF.Sqrt, bias=EPS_BN)
            rs = cst.tile([128, 1], F32, name=f"{name}_rs")
            nc.vector.reciprocal(rs[:], sd[:])
            scale = cst.tile([128, 1], F32, name=f"{name}_scale")
            nc.vector.tensor_tensor(out=scale[:], in0=g_ap, in1=rs[:], op=OP.mult)
            shift = cst.tile([128, 1], F32, name=f"{name}_shift")
            nc.vector.tensor_tensor(out=shift[:], in0=mean[:], in1=scale[:],
                                    op=OP.mult)
            nc.vector.tensor_tensor(out=shift[:], in0=beta_ap, in1=shift[:],
                                    op=OP.subtract)
            return scale, shift

        def allreduce(sb_in_ap, width, name):
            bi = dr.tile([128, width], F32, name=f"{name}_in")
            bo = dr.tile([128, width], F32, name=f"{name}_out")
            nc.sync.dma_start(bi[:], sb_in_ap)
            nc.gpsimd.collective_compute(
                "AllReduce", OP.add, replica_groups=[list(range(NCORES))],
                ins=[bi[:].opt()], outs=[bo[:].opt()],
            )
            st = cst.tile([128, width], F32, name=f"{name}_st")
            nc.sync.dma_start(st[:], bo[:])
            return st

        # ---- phase 1: BN0 stats over x (empty slots are exact zeros)
        s0c = cst.tile([128, nblk], F32, name="s0c")
        q0c = cst.tile([128, nblk], F32, name="q0c")
        for i, (o, w) in enumerate(blocks):
            scr = wk.tile([128, 512], F32, name="scr0", tag="scratch")
            nc.scalar.activation(scr[:, :w], xT[:, o:o + w], AF.Square,
                                 accum_out=q0c[:, i:i + 1])
            nc.vector.tensor_reduce(s0c[:, i:i + 1], xT[:, o:o + w],
                                    axis=mybir.AxisListType.X, op=OP.add)
        st0in = cst.tile([128, 2], F32, name="st0in")
        nc.vector.tensor_reduce(st0in[:, 0:1], s0c[:],
                                axis=mybir.AxisListType.X, op=OP.add)
        nc.vector.tensor_reduce(st0in[:, 1:2], q0c[:],
                                axis=mybir.AxisListType.X, op=OP.add)
        st0 = allreduce(st0in[:], 2, "ar0")
        scale0, shift0 = bn_params(st0[:, 0:1], st0[:, 1:2], g0, b0, "bn0")

        # ---- phase 2: h (transposed + row table + allgather)
        h_cT = big.tile([128, nloc], F32, name="h_cT", tag="slotA")
        for o, w in blocks:
            nc.scalar.activation(h_cT[:, o:o + w], xT[:, o:o + w], AF.Relu,
                                 bias=shift0[:, 0:1], scale=scale0[:, 0:1])
        h_shard = dr.tile([nloc, 128], F32, name="h_shard")
        for t in range(tiles):
            trp = ps.tile([128, 128], F32, name="trp_h", tag="trp")
            nc.tensor.transpose(trp[:], h_cT[:, t * 128:(t + 1) * 128], ident[:])
            hrow = wk.tile([128, 128], F32, name="hrow", tag="hrow", bufs=3)
            nc.scalar.copy(hrow[:], trp[:])
            nc.sync.dma_start(h_shard[t * 128:(t + 1) * 128, :], hrow[:])
        h_full = dr.tile([ntab, 128], F32, name="h_full", addr_space="Shared")
        nc.gpsimd.collective_compute(
            "AllGather", OP.bypass, replica_groups=[list(range(NCORES))],
            ins=[h_shard[:].opt()], outs=[h_full[:].opt()],
        )

        # ---- phase 3: edge aggregation (per-tile batched gather + bf16 mm)
        z0 = big.tile([128, nloc], F32R, name="z0", tag="slotB")
        for t in range(tiles):
            bt = int(B[t])
            col0 = int(colofs[t])
            psum = ps.tile([128, 256], F32, name="epsum", tag="edge_psum",
                           bufs=3)
            gbuf = wk.tile([128, bt * 128], F32, name="gbuf", tag="gbuf",
                           bufs=4)
            for b in range(bt):
                nc.gpsimd.indirect_dma_start(
                    out=gbuf[:, b * 128:(b + 1) * 128], out_offset=None,
                    in_=h_full[:],
                    in_offset=bass.IndirectOffsetOnAxis(
                        ap=esrc[:, col0 + b:col0 + b + 1], axis=0),
                )
            V = wk.tile([128, bt * 256], BF16, name="V", tag="V", bufs=2)
            S = wk.tile([128, bt * 128], BF16, name="S", tag="S", bufs=2)
            gb3 = gbuf[:].rearrange("p (b c) -> p b c", c=128)
            V3 = V[:].rearrange("p (b c) -> p b c", c=256)
            nc.scalar.activation(V3[:, :, 0:128], gb3[:], AF.Exp,
                                 bias=EPS_GEN)
            nc.vector.tensor_tensor(out=V3[:, :, 128:256], in0=gb3[:],
                                    in1=V3[:, :, 0:128], op=OP.mult)
            iota_b = iota[:].rearrange(
                "p (g c) -> p g c", g=1).to_broadcast([128, bt, 128])
            dst_b = edst[:, col0:col0 + bt].rearrange(
                "p (b o) -> p b o", o=1).to_broadcast([128, bt, 128])
            nc.vector.tensor_tensor(
                out=S[:].rearrange("p (b c) -> p b c", c=128),
                in0=iota_b, in1=dst_b, op=OP.is_equal)
            for b in range(bt):
                nc.tensor.matmul(psum[:],
                                 lhsT=S[:, b * 128:(b + 1) * 128],
                                 rhs=V[:, b * 256:(b + 1) * 256],
                                 start=(b == 0), stop=(b == bt - 1))
            denc = wk.tile([128, 128], F32, name="denc", tag="denc", bufs=3)
            nc.vector.tensor_scalar(out=denc[:], in0=psum[:, 0:128],
                                    scalar1=DEN_EPS, scalar2=None,
                                    op0=OP.max)
            rec = wk.tile([128, 128], F32, name="rec", tag="rec", bufs=3)
            nc.vector.reciprocal(rec[:], denc[:])
            agg = wk.tile([128, 128], F32, name="agg", tag="agg", bufs=3)
            nc.vector.tensor_tensor(out=agg[:], in0=psum[:, 128:256],
                                    in1=rec[:], op=OP.mult)
            trp2 = ps.tile([128, 128], F32, name="trp_a", tag="trp")
            nc.tensor.transpose(trp2[:], agg[:], ident[:])
            nc.vector.tensor_tensor(
                out=z0[:, t * 128:(t + 1) * 128], in0=trp2[:],
                in1=h_cT[:, t * 128:(t + 1) * 128], op=OP.add)

        # ---- phases 4..6: MLP layer helper
        def mlp_layer(zin_list, wtile, wofs, kparts, zout_tags, b_aps, g_aps,
                      be_aps, name):
            """zout = relu(BN(sum_k W[k]^T @ zin[k] + b)) with empty-slot
            corrected global BN. Returns list of big tiles (2 halves for
            H-out, 1 for C-out)."""
            mparts = len(zout_tags)
            zraw = [big.tile([128, nloc], F32, name=f"{name}_raw{m}",
                             tag=zout_tags[m][0]) for m in range(mparts)]
            ssum = [cst.tile([128, nblk], F32, name=f"{name}_ss{m}")
                    for m in range(mparts)]
            sssq = [cst.tile([128, nblk], F32, name=f"{name}_sq{m}")
                    for m in range(mparts)]
            for i, (o, w) in enumerate(blocks):
                for m in range(mparts):
                    pmm = ps.tile([128, 512], F32, name=f"{name}_ps",
                                  tag="mm", bufs=3)
                    for k in range(kparts):
                        lhs = wtile[:, wofs(k, m):wofs(k, m) + 128]
                        nc.tensor.matmul(
                            pmm[:, :w], lhsT=lhs,
                            rhs=zin_list[k][:, o:o + w],
                            start=(k == 0), stop=(k == kparts - 1))
                    nc.scalar.activation(zraw[m][:, o:o + w], pmm[:, :w],
                                         AF.Identity, bias=b_aps[m],
                                         accum_out=ssum[m][:, i:i + 1])
                    scr = wk.tile([128, 512], F32, name=f"{name}_scr",
                                  tag="scratch")
                    nc.scalar.activation(scr[:, :w], zraw[m][:, o:o + w],
                                         AF.Square,
                                         accum_out=sssq[m][:, i:i + 1])
            # stats + empty-slot correction
            arin = cst.tile([128, 2 * mparts], F32, name=f"{name}_arin")
            for m in range(mparts):
                rs_ = cst.tile([128, 2], F32, name=f"{name}_r{m}")
                nc.vector.tensor_reduce(rs_[:, 0:1], ssum[m][:],
                                        axis=mybir.AxisListType.X, op=OP.add)
                nc.vector.tensor_reduce(rs_[:, 1:2], sssq[m][:],
                                        axis=mybir.AxisListType.X, op=OP.add)
                delta = zraw[m][:, nloc - 1:nloc]
                t1 = cst.tile([128, 1], F32, name=f"{name}_t1{m}")
                nc.vector.tensor_tensor(out=t1[:], in0=delta, in1=n_emp,
                                        op=OP.mult)
                nc.vector.tensor_tensor(out=arin[:, m:m + 1], in0=rs_[:, 0:1],
                                        in1=t1[:], op=OP.subtract)
                d2 = cst.tile([128, 1], F32, name=f"{name}_d2{m}")
                nc.vector.tensor_tensor(out=d2[:], in0=delta, in1=delta,
                                        op=OP.mult)
                nc.vector.tensor_tensor(out=d2[:], in0=d2[:], in1=n_emp,
                                        op=OP.mult)
                nc.vector.tensor_tensor(out=arin[:, mparts + m:mparts + m + 1],
                                        in0=rs_[:, 1:2], in1=d2[:],
                                        op=OP.subtract)
            st = allreduce(arin[:], 2 * mparts, f"{name}_ar")
            zout = []
            for m in range(mparts):
                sc, sh = bn_params(st[:, m:m + 1],
                                   st[:, mparts + m:mparts + m + 1],
                                   g_aps[m], be_aps[m], f"{name}_p{m}")
                zo = big.tile([128, nloc], F32R, name=f"{name}_n{m}",
                              tag=zout_tags[m][1])
                for o, w in blocks:
                    nc.scalar.activation(zo[:, o:o + w], zraw[m][:, o:o + w],
                                         AF.Relu, bias=sh[:, 0:1],
                                         scale=sc[:, 0:1])
                zout.append(zo)
            return zout

        # L1: z0 [C,n] -> z1 halves; W1 [128, 2H]: lhsT for half m = w1[:, m*128...]
        z1 = mlp_layer([z0], w1, lambda k, m: m * 128, 1,
                       [("slotA", "slotB"), ("slotC", "slotD")],
                       b1, g1, be1, "l1")
        # L2: z1 (2 K-parts) -> z2 halves; W2sb [128, 512]:
        # lhsT(k,m) at col k*256 + m*128
        z2 = mlp_layer(z1, w2, lambda k, m: k * 256 + m * 128, 2,
                       [("slotA", "slotB"), ("slotC", "slotD")],
                       b2, g2, be2, "l2")

        # ---- phase 7: z3 = W3^T @ z2 + b3; u = z3 + xT; out = We^T @ u + be
        for o, w in blocks:
            ps3 = ps.tile([128, 512], F32, name="ps3", tag="mm", bufs=3)
            for k in range(2):
                nc.tensor.matmul(
                    ps3[:, :w], lhsT=w3[:, k * 128:k * 128 + 128],
                    rhs=z2[k][:, o:o + w],
                    start=(k == 0), stop=(k == 1))
            u = wk.tile([128, 512], F32R, name="u", tag="u", bufs=3)
            nc.scalar.activation(u[:, :w], ps3[:, :w], AF.Identity, bias=b3)
            nc.vector.tensor_tensor(out=u[:, :w], in0=u[:, :w],
                                    in1=xT[:, o:o + w], op=OP.add)
            ps4 = ps.tile([128, 512], F32, name="ps4", tag="mm", bufs=3)
            nc.tensor.matmul(ps4[:, :w], lhsT=we[:], rhs=u[:, :w],
                             start=True, stop=True)
            ob = wk.tile([128, 512], F32, name="ob", tag="ob", bufs=3)
            nc.scalar.activation(ob[:, :w], ps4[:, :w], AF.Identity,
                                 bias=be_enc)
            nc.sync.dma_start(out_d[:, o:o + w], ob[:, :w])

    nc.compile()
    return nc


# --------------------------------------------------------------------------
# public entry
# --------------------------------------------------------------------------

_CACHE = {}
LAST_RESULT = None


def _run(x, edge_index, bn_g, bn_b, W1, b1, g1, be1, W2, b2, g2, be2,
         W3, b3, We, be, n_nodes):
    src = np.asarray(edge_index[0], dtype=np.int64)
    dst = np.asarray(edge_index[1], dtype=np.int64)
    x = np.asarray(x, dtype=np.float32)

    plan = _plan(n_nodes, src, dst)
    tiles, nloc, NB = plan["tiles"], plan["nloc"], plan["NB"]

    key = (tiles, NB, tuple(plan["B"].tolist()), n_nodes)
    if key not in _CACHE:
        _CACHE[key] = _build(plan, n_nodes)
    nc = _CACHE[key]

    h_dim = W1.shape[1]
    # pack weights
    w2sb = np.ascontiguousarray(
        W2.reshape(2, 128, h_dim).transpose(1, 0, 2).reshape(128, 2 * h_dim)
    ).astype(np.float32)
    w3sb = np.ascontiguousarray(
        W3.reshape(2, 128, 128).transpose(1, 0, 2).reshape(128, 256)
    ).astype(np.float32)
    iota = np.broadcast_to(np.arange(128, dtype=np.float32), (128, 128)).copy()

    halves = lambda v: [np.asarray(v[:128], np.float32),
                        np.asarray(v[128:], np.float32)]
    b1h, g1h, be1h = halves(b1), halves(g1), halves(be1)
    b2h, g2h, be2h = halves(b2), halves(g2), halves(be2)

    in_maps = []
    core_nodes = []
    for c in range(NCORES):
        sel = plan["core_of"] == c
        nodes_c = np.nonzero(sel)[0]
        core_nodes.append(nodes_c)
        xr = np.zeros((nloc, 128), np.float32)
        xr[plan["loc_of"][nodes_c]] = x[nodes_c]
        pcols = np.zeros((128, 17), np.float32)
        pcols[:, 0] = bn_g
        pcols[:, 1] = bn_b
        for i2 in range(2):
            pcols[:, 2 + i2] = b1h[i2]
            pcols[:, 4 + i2] = g1h[i2]
            pcols[:, 6 + i2] = be1h[i2]
            pcols[:, 8 + i2] = b2h[i2]
            pcols[:, 10 + i2] = g2h[i2]
            pcols[:, 12 + i2] = be2h[i2]
        pcols[:, 14] = b3
        pcols[:, 15] = be
        pcols[:, 16] = plan["n_empty"][c]
        in_maps.append({
            "xT": np.ascontiguousarray(xr.T),
            "esrc": plan["esrc"][c],
            "edst": plan["edst"][c],
            "iota": iota,
            "w1": np.asarray(W1, np.float32),
            "w2": w2sb,
            "w3": w3sb,
            "we": np.asarray(We, np.float32),
            "pcols": pcols,
        })

    import os
    trace = bool(os.environ.get("KTRACE"))
    res = run_bass_kernel_spmd(nc, in_maps, list(range(NCORES)), trace=trace)
    global LAST_RESULT
    LAST_RESULT = res
    out = np.empty((n_nodes, 128), np.float32)
    for c in range(NCORES):
        nodes_c = core_nodes[c]
        out[nodes_c] = res.results[c]["outT"][:, plan["loc_of"][nodes_c]].T
    return out


def kernel(x, edge_index, bn_g, bn_b, W1, b1, g1, be1, W2, b2, g2, be2,
           W3, b3, We, be):
    return _run(x, edge_index, bn_g, bn_b, W1, b1, g1, be1, W2, b2, g2, be2,
                W3, b3, We, be, n_nodes=x.shape[0])



# revision 6
# speedup vs baseline: 1.1484x; 1.1484x over previous
"""Trainium2 Bass kernel for DeepGCNLayer(GENConv softmax-aggr) + encoder.

Computation (see reference):
  h  = relu(batchnorm(x))                       # BN0 over all N nodes
  msg_e = relu(h[src_e]) + eps = h[src_e] + eps # h >= 0 already
  agg_v = softmax-weighted mean of msg over incoming edges (t=1)
  z0 = agg + h
  z1 = relu(BN1(z0 @ W1 + b1)); z2 = relu(BN2(z1 @ W2 + b2))
  out = (x + z2 @ W3 + b3) @ We + be

Strategy (8 NeuronCores, SPMD single program):
  * Host packs nodes into 128-slot tiles balanced by in-degree, tiles
    assigned to cores; per-core activations live transposed [ch, nodes].
  * BN stats computed on-device, AllReduce'd (tiny f32 cols).
  * h computed per-shard in bf16, AllGather'd into a replicated row
    table viewed as node PAIRS [ntab/2, 256] so dma_gather's int16
    indices (prow>>1 < 32768) can address it.
  * Edge phase: per node tile, ONE InstDMAGatherAnt gathers all edge
    source pairs (512B each); edges are pre-sorted by (prow&1) so the
    pair half each block needs is static. exp on ACT, one-hot selection
    matrix (DVE compare vs iota, bf16), PE matmul S^T @ [e | m*e]
    accumulated in PSUM per node tile.
  * MLP phase: bf16 weight-stationary PE matmuls over 512-col node
    blocks; raw pass on ACT with accum_out sums; squares via big DVE
    instructions; BN relu applied in one big ACT instruction per half.
  * Empty node slots contribute a constant column; corrected in BN
    stats using the reserved always-empty last slot times n_empty.
"""

import math
import numpy as np
from contextlib import ExitStack

import concourse.bass as bass
import concourse.tile as tile
from concourse import bacc, mybir, library_config
from concourse.bass_utils import run_bass_kernel_spmd
from concourse.masks import make_identity

F32 = mybir.dt.float32
F32R = mybir.dt.float32r
BF16 = mybir.dt.bfloat16
I16 = mybir.dt.int16
AF = mybir.ActivationFunctionType
OP = mybir.AluOpType

NCORES = 8
EPS_GEN = 1e-7
EPS_BN = 1e-5
DEN_EPS = 1e-16

# full-size problem config (hardcoded; harness calls kernel() directly)
N_FULL = 50000
C = 128
H = 256


# --------------------------------------------------------------------------
# host-side graph partitioning
# --------------------------------------------------------------------------

def _plan(n_nodes, src, dst):
    """Pack nodes into 128-slot bins balanced by in-degree; assign bins to
    cores; build per-core edge blocks (sorted by src-pair half) and
    permutation arrays."""
    import heapq

    tiles = math.ceil(n_nodes / (NCORES * 128) + 0.02)  # slack for reserve
    if tiles * NCORES * 128 - NCORES < n_nodes:
        tiles += 1
    nbins = NCORES * tiles
    nloc = tiles * 128

    deg = np.bincount(dst, minlength=n_nodes).astype(np.int64)
    order = np.argsort(-deg, kind="stable")
    heap = [(0, i) for i in range(nbins)]
    heapq.heapify(heap)
    bin_nodes = [[] for _ in range(nbins)]
    bin_load = np.zeros(nbins, np.int64)
    for g in order:
        d = int(deg[g])
        spill = []
        while True:
            load, b = heapq.heappop(heap)
            if len(bin_nodes[b]) < 128:
                bin_nodes[b].append(g)
                bin_load[b] = load + d
                heapq.heappush(heap, (load + d, b))
                break
            spill.append((load, b))
        for it in spill:
            heapq.heappush(heap, it)

    # snake-assign bins to cores by load
    border = np.argsort(-bin_load, kind="stable")
    core_bins = [[] for _ in range(NCORES)]
    for i, b in enumerate(border):
        r = i // NCORES
        c = i % NCORES if r % 2 == 0 else NCORES - 1 - (i % NCORES)
        core_bins[c].append(b)
    # per-core: order bins by load desc; reserved empty slot = last slot of
    # last tile -> last bin must have <= 127 nodes
    for c in range(NCORES):
        core_bins[c].sort(key=lambda b: -bin_load[b])
        last = core_bins[c][-1]
        if len(bin_nodes[last]) >= 128:
            g = bin_nodes[last].pop()  # move lowest-degree node
            moved = False
            for b in core_bins[c]:
                if b != last and len(bin_nodes[b]) < 128:
                    bin_nodes[b].append(g)
                    moved = True
                    break
            if not moved:
                for c2 in range(NCORES):
                    for b in core_bins[c2]:
                        if b != last and len(bin_nodes[b]) < 128:
                            bin_nodes[b].append(g)
                            moved = True
                            break
                    if moved:
                        break
            assert moved

    core_of = np.empty(n_nodes, np.int64)
    tile_of = np.empty(n_nodes, np.int64)
    slot_of = np.empty(n_nodes, np.int64)
    for c in range(NCORES):
        for k, b in enumerate(core_bins[c]):
            for s, g in enumerate(bin_nodes[b]):
                core_of[g] = c
                tile_of[g] = k
                slot_of[g] = s
    loc_of = tile_of * 128 + slot_of
    prow = core_of * nloc + loc_of

    # per (core, tile, half) edge loads -> uniform block counts A0/A1
    half = prow[src] & 1                       # pair half of the SOURCE
    ecore = core_of[dst]
    etile = tile_of[dst]
    key2 = (ecore * tiles + etile) * 2 + half
    counts2 = np.bincount(key2, minlength=nbins * 2).reshape(NCORES, tiles, 2)
    A0 = np.maximum(1, (counts2[:, :, 0].max(axis=0) + 127) // 128)
    A1 = np.maximum(1, (counts2[:, :, 1].max(axis=0) + 127) // 128)
    B = (A0 + A1).astype(np.int64)
    colofs = np.concatenate([[0], np.cumsum(B)])
    NB = int(colofs[-1])

    # build edge arrays: per (core,tile): half0 edges at slots [0, n0),
    # half1 edges at slots [A0*128, A0*128+n1); pads idx=0 edst=255
    idx16 = np.zeros((NCORES, 128, NB), np.int16)  # [lane, col] layout
    edst = np.full((NCORES, 128, NB), 255.0, np.float32)
    ekey = (ecore * tiles + etile) * 2 + half
    eorder = np.argsort(ekey, kind="stable")
    sk = ekey[eorder]
    starts = np.concatenate(
        [[0], np.cumsum(np.bincount(sk, minlength=nbins * 2))])
    pos = np.arange(len(sk)) - starts[sk]      # rank within (core,tile,half)
    e_half = sk % 2
    e_ct = sk // 2
    e_c = e_ct // tiles
    e_t = e_ct % tiles
    slotbase = np.where(e_half == 0, 0, A0[e_t] * 128)
    slot = slotbase + pos                       # slot within tile
    lane = slot % 128
    col = colofs[e_t] + slot // 128
    idx16[e_c, lane, col] = (prow[src[eorder]] >> 1).astype(np.int16)
    edst[e_c, lane, col] = slot_of[dst[eorder]].astype(np.float32)

    # wrap idx16 into dma_gather layout: logical k=(col-colofs)*128+lane ->
    # [k%16, colofs8 + k//16], replicated over 8 partition groups
    colofs8 = colofs * 8
    NBI = NB * 8
    idxw = np.zeros((NCORES, 128, NBI), np.int16)
    for t in range(tiles):
        bt = int(B[t])
        blk = idx16[:, :, colofs[t]:colofs[t] + bt]        # [C,128,bt]
        seq = blk.transpose(0, 2, 1).reshape(NCORES, bt * 128)  # k=col*128+lane
        wrap = seq.reshape(NCORES, bt * 8, 16).transpose(0, 2, 1)  # [C,16,bt*8]
        idxw[:, :, colofs8[t]:colofs8[t] + bt * 8] = np.tile(wrap, (1, 8, 1))

    n_empty = np.array(
        [nloc - sum(len(bin_nodes[b]) for b in core_bins[c]) for c in range(NCORES)],
        np.float32,
    )
    return dict(
        tiles=tiles, nloc=nloc, ntab=NCORES * nloc, NB=NB, NBI=NBI,
        B=B, A0=A0, A1=A1, colofs=colofs, colofs8=colofs8,
        core_of=core_of, loc_of=loc_of, prow=prow,
        idxw=idxw, edst=edst, n_empty=n_empty,
    )


def _node_blocks(nloc):
    out = []
    o = 0
    while o < nloc:
        w = min(512, nloc - o)
        out.append((o, w))
        o += w
    return out


def _reg_const(nc, val, dtype=F32):
    t = nc.alloc_sbuf_tensor(f"constap-{val}", [128, 1], dtype)
    nc.gpsimd.memset(t.ap(), val)
    nc.const_aps.aps[(dtype, val)] = t.ap()


# --------------------------------------------------------------------------
# kernel build
# --------------------------------------------------------------------------

def _build(plan, n_real):
    tiles, nloc, ntab = plan["tiles"], plan["nloc"], plan["ntab"]
    NB, NBI = plan["NB"], plan["NBI"]
    B, A0, colofs, colofs8 = plan["B"], plan["A0"], plan["colofs"], plan["colofs8"]
    blocks = _node_blocks(nloc)
    nblk = len(blocks)
    inv_n = 1.0 / float(n_real)

    nc = bacc.Bacc("TRN2", target_bir_lowering=False, debug=False,
                   num_devices=NCORES)
    for v in (EPS_GEN, EPS_BN, DEN_EPS):
        _reg_const(nc, v)
    nc.all_engine_barrier()

    xT_d = nc.declare_dram_parameter("xT", [128, nloc], F32, isOutput=False)
    idx_d = nc.declare_dram_parameter("idx16", [128, NBI], I16, isOutput=False)
    edst_d = nc.declare_dram_parameter("edst", [128, NB], BF16, isOutput=False)
    iota_d = nc.declare_dram_parameter("iota", [128, 128], BF16, isOutput=False)
    w1_d = nc.declare_dram_parameter("w1", [128, H], BF16, isOutput=False)
    w2_d = nc.declare_dram_parameter("w2", [128, 2 * H], BF16, isOutput=False)
    w3_d = nc.declare_dram_parameter("w3", [128, H], BF16, isOutput=False)
    we_d = nc.declare_dram_parameter("we", [128, C], F32R, isOutput=False)
    pc_d = nc.declare_dram_parameter("pcols", [128, 17], F32, isOutput=False)
    out_d = nc.declare_dram_parameter("outT", [128, nloc], F32, isOutput=True)

    with ExitStack() as ctx:
        tc = ctx.enter_context(tile.TileContext(nc))
        cst = ctx.enter_context(tc.tile_pool(name="cst", bufs=1))
        big = ctx.enter_context(tc.tile_pool(name="big", bufs=1))
        wk = ctx.enter_context(tc.tile_pool(name="wk", bufs=4))
        ps = ctx.enter_context(tc.tile_pool(name="ps", bufs=2, space="PSUM"))
        dr = ctx.enter_context(tc.tile_pool(name="dr", bufs=1, space="DRAM"))

        nc.gpsimd.load_library(library_config.mlp)

        # ---- load constants / params
        xT = big.tile([128, nloc], F32, name="xT", tag="slot_xT")
        nc.sync.dma_start(xT[:], xT_d[:])
        idx16 = cst.tile([128, NBI], I16, name="idx16")
        nc.sync.dma_start(idx16[:], idx_d[:])
        edst = cst.tile([128, NB], BF16, name="edst")
        nc.sync.dma_start(edst[:], edst_d[:])
        iota = cst.tile([128, 128], BF16, name="iota")
        nc.sync.dma_start(iota[:], iota_d[:])
        w1 = cst.tile([128, H], BF16, name="w1")
        nc.sync.dma_start(w1[:], w1_d[:])
        w2 = cst.tile([128, 2 * H], BF16, name="w2")
        nc.sync.dma_start(w2[:], w2_d[:])
        w3 = cst.tile([128, H], BF16, name="w3")
        nc.sync.dma_start(w3[:], w3_d[:])
        we = cst.tile([128, C], F32R, name="we")
        nc.sync.dma_start(we[:], we_d[:])
        pc = cst.tile([128, 17], F32, name="pc")
        nc.sync.dma_start(pc[:], pc_d[:])
        ident = cst.tile([128, 128], F32, name="ident")
        make_identity(nc, ident[:])

        g0, b0 = pc[:, 0:1], pc[:, 1:2]
        b1 = [pc[:, 2:3], pc[:, 3:4]]
        g1 = [pc[:, 4:5], pc[:, 5:6]]
        be1 = [pc[:, 6:7], pc[:, 7:8]]
        b2 = [pc[:, 8:9], pc[:, 9:10]]
        g2 = [pc[:, 10:11], pc[:, 11:12]]
        be2 = [pc[:, 12:13], pc[:, 13:14]]
        b3, be_enc, n_emp = pc[:, 14:15], pc[:, 15:16], pc[:, 16:17]

        # ---- helper: BN params from allreduced [sum, sumsq] cols
        def bn_params(st_sum, st_ssq, g_ap, beta_ap, name):
            mean = cst.tile([128, 1], F32, name=f"{name}_mean")
            nc.vector.tensor_scalar(out=mean[:], in0=st_sum, scalar1=inv_n,
                                    scalar2=None, op0=OP.mult)
            msq = cst.tile([128, 1], F32, name=f"{name}_msq")
            nc.vector.tensor_scalar(out=msq[:], in0=st_ssq, scalar1=inv_n,
                                    scalar2=None, op0=OP.mult)
            var = cst.tile([128, 1], F32, name=f"{name}_var")
            nc.vector.tensor_tensor(out=var[:], in0=mean[:], in1=mean[:],
                                    op=OP.mult)
            nc.vector.tensor_tensor(out=var[:], in0=msq[:], in1=var[:],
                                    op=OP.subtract)
            sd = cst.tile([128, 1], F32, name=f"{name}_sd")
            nc.scalar.activation(sd[:], var[:], AF.Sqrt, bias=EPS_BN)
            rs = cst.tile([128, 1], F32, name=f"{name}_rs")
            nc.vector.reciprocal(rs[:], sd[:])
            scale = cst.tile([128, 1], F32, name=f"{name}_scale")
            nc.vector.tensor_tensor(out=scale[:], in0=g_ap, in1=rs[:], op=OP.mult)
            shift = cst.tile([128, 1], F32, name=f"{name}_shift")
            nc.vector.tensor_tensor(out=shift[:], in0=mean[:], in1=scale[:],
                                    op=OP.mult)
            nc.vector.tensor_tensor(out=shift[:], in0=beta_ap, in1=shift[:],
                                    op=OP.subtract)
            return scale, shift

        def allreduce(sb_in_ap, width, name):
            bi = dr.tile([128, width], F32, name=f"{name}_in")
            bo = dr.tile([128, width], F32, name=f"{name}_out")
            nc.sync.dma_start(bi[:], sb_in_ap)
            nc.gpsimd.collective_compute(
                "AllReduce", OP.add, replica_groups=[list(range(NCORES))],
                ins=[bi[:].opt()], outs=[bo[:].opt()],
            )
            st = cst.tile([128, width], F32, name=f"{name}_st")
            nc.sync.dma_start(st[:], bo[:])
            return st

        # square scratch (bf16, values discarded; only reductions used)
        scr = big.tile([128, nloc], BF16, name="scr", tag="slotS")

        # ---- phase 1: BN0 stats over x (empty slots are exact zeros)
        st0in = cst.tile([128, 2], F32, name="st0in")
        nc.vector.tensor_reduce(st0in[:, 0:1], xT[:],
                                axis=mybir.AxisListType.X, op=OP.add)
        nc.vector.tensor_tensor(out=scr[:], in0=xT[:], in1=xT[:], op=OP.mult)
        nc.vector.tensor_reduce(st0in[:, 1:2], scr[:],
                                axis=mybir.AxisListType.X, op=OP.add)
        st0 = allreduce(st0in[:], 2, "ar0")
        scale0, shift0 = bn_params(st0[:, 0:1], st0[:, 1:2], g0, b0, "bn0")

        # ---- phase 2: h (transposed layout + bf16 row table + allgather)
        h_cT = big.tile([128, nloc], F32, name="h_cT", tag="slotA")
        nc.scalar.activation(h_cT[:], xT[:], AF.Relu,
                             bias=shift0[:, 0:1], scale=scale0[:, 0:1])
        h_shard = dr.tile([nloc // 2, 256], BF16, name="h_shard")
        for t in range(tiles):
            trp = ps.tile([128, 128], F32, name="trp_h", tag="trp")
            nc.tensor.transpose(trp[:], h_cT[:, t * 128:(t + 1) * 128], ident[:])
            hrow = wk.tile([128, 128], BF16, name="hrow", tag="hrow", bufs=3)
            nc.scalar.copy(hrow[:], trp[:])
            dst_view = h_shard[t * 64:(t + 1) * 64, :].rearrange(
                "a (two c) -> (a two) c", two=2)
            nc.sync.dma_start(dst_view, hrow[:])
        h_full = dr.tile([ntab // 2, 256], BF16, name="h_full",
                         addr_space="Shared")
        nc.gpsimd.collective_compute(
            "AllGather", OP.bypass, replica_groups=[list(range(NCORES))],
            ins=[h_shard[:].opt()], outs=[h_full[:].opt()],
        )

        # ---- phase 3: edge aggregation (one dma_gather per tile + bf16 mm)
        z0 = big.tile([128, nloc], BF16, name="z0", tag="slotB")
        for t in range(tiles):
            bt = int(B[t])
            a0 = int(A0[t])
            col0 = int(colofs[t])
            c8 = int(colofs8[t])
            psum = ps.tile([128, 256], F32, name="epsum", tag="edge_psum",
                           bufs=3)
            gbuf = wk.tile([128, bt, 256], BF16, name="gbuf", tag="gbuf",
                           bufs=2)
            # HW caps one dma_gather at 1024 idxs (64 int16 cols/partition)
            for go in range(0, bt, 8):
                gw = min(8, bt - go)
                nc.gpsimd.dma_gather(
                    gbuf[:, go:go + gw, :], h_full[:],
                    idx16[:, c8 + go * 8:c8 + (go + gw) * 8],
                    gw * 128, gw * 128, 256)
            V = wk.tile([128, bt, 256], BF16, name="V", tag="V", bufs=2)
            # half0 blocks read pair half [0:128]; half1 blocks [128:256]
            if a0 > 0:
                nc.scalar.activation(V[:, 0:a0, 0:128], gbuf[:, 0:a0, 0:128],
                                     AF.Exp, bias=EPS_GEN)
                nc.vector.tensor_tensor(out=V[:, 0:a0, 128:256],
                                        in0=gbuf[:, 0:a0, 0:128],
                                        in1=V[:, 0:a0, 0:128], op=OP.mult)
            if a0 < bt:
                nc.scalar.activation(V[:, a0:bt, 0:128],
                                     gbuf[:, a0:bt, 128:256],
                                     AF.Exp, bias=EPS_GEN)
                nc.vector.tensor_tensor(out=V[:, a0:bt, 128:256],
                                        in0=gbuf[:, a0:bt, 128:256],
                                        in1=V[:, a0:bt, 0:128], op=OP.mult)
            S = wk.tile([128, bt, 128], BF16, name="S", tag="S", bufs=2)
            iota_b = iota[:].rearrange(
                "p (g c) -> p g c", g=1).to_broadcast([128, bt, 128])
            dst_b = edst[:, col0:col0 + bt].rearrange(
                "p (b o) -> p b o", o=1).to_broadcast([128, bt, 128])
            nc.vector.tensor_tensor(out=S[:], in0=iota_b, in1=dst_b,
                                    op=OP.is_equal)
            for b in range(bt):
                nc.tensor.matmul(psum[:],
                                 lhsT=S[:, b:b + 1, :],
                                 rhs=V[:, b:b + 1, :],
                                 start=(b == 0), stop=(b == bt - 1))
            denc = wk.tile([128, 128], F32, name="denc", tag="denc", bufs=3)
            nc.vector.tensor_scalar(out=denc[:], in0=psum[:, 0:128],
                                    scalar1=DEN_EPS, scalar2=None,
                                    op0=OP.max)
            rec = wk.tile([128, 128], F32, name="rec", tag="rec", bufs=3)
            nc.vector.reciprocal(rec[:], denc[:])
            agg = wk.tile([128, 128], F32, name="agg", tag="agg", bufs=3)
            nc.vector.tensor_tensor(out=agg[:], in0=psum[:, 128:256],
                                    in1=rec[:], op=OP.mult)
            trp2 = ps.tile([128, 128], F32, name="trp_a", tag="trp")
            nc.tensor.transpose(trp2[:], agg[:], ident[:])
            nc.vector.tensor_tensor(
                out=z0[:, t * 128:(t + 1) * 128], in0=trp2[:],
                in1=h_cT[:, t * 128:(t + 1) * 128], op=OP.add)

        # ---- phases 4..6: MLP layer helper
        def mlp_layer(zin_list, wtile, wofs, kparts, zout_tags, b_aps, g_aps,
                      be_aps, name):
            """zout = relu(BN(sum_k W[k]^T @ zin[k] + b)) with empty-slot
            corrected global BN. zraw halves in f32 big slots; zout bf16."""
            mparts = len(zout_tags)
            zraw = [big.tile([128, nloc], F32, name=f"{name}_raw{m}",
                             tag=zout_tags[m][0]) for m in range(mparts)]
            ssum = [cst.tile([128, nblk], F32, name=f"{name}_ss{m}")
                    for m in range(mparts)]
            for i, (o, w) in enumerate(blocks):
                for m in range(mparts):
                    pmm = ps.tile([128, 512], F32, name=f"{name}_ps",
                                  tag="mm", bufs=3)
                    for k in range(kparts):
                        lhs = wtile[:, wofs(k, m):wofs(k, m) + 128]
                        nc.tensor.matmul(
                            pmm[:, :w], lhsT=lhs,
                            rhs=zin_list[k][:, o:o + w],
                            start=(k == 0), stop=(k == kparts - 1))
                    nc.scalar.activation(zraw[m][:, o:o + w], pmm[:, :w],
                                         AF.Identity, bias=b_aps[m],
                                         accum_out=ssum[m][:, i:i + 1])
            # stats: big DVE square + reduce per half, empty-slot corrected
            arin = cst.tile([128, 2 * mparts], F32, name=f"{name}_arin")
            for m in range(mparts):
                rs_ = cst.tile([128, 2], F32, name=f"{name}_r{m}")
                nc.vector.tensor_reduce(rs_[:, 0:1], ssum[m][:],
                                        axis=mybir.AxisListType.X, op=OP.add)
                nc.vector.tensor_tensor(out=scr[:], in0=zraw[m][:],
                                        in1=zraw[m][:], op=OP.mult)
                nc.vector.tensor_reduce(rs_[:, 1:2], scr[:],
                                        axis=mybir.AxisListType.X, op=OP.add)
                delta = zraw[m][:, nloc - 1:nloc]
                t1 = cst.tile([128, 1], F32, name=f"{name}_t1{m}")
                nc.vector.tensor_tensor(out=t1[:], in0=delta, in1=n_emp,
                                        op=OP.mult)
                nc.vector.tensor_tensor(out=arin[:, m:m + 1], in0=rs_[:, 0:1],
                                        in1=t1[:], op=OP.subtract)
                d2 = cst.tile([128, 1], F32, name=f"{name}_d2{m}")
                nc.vector.tensor_tensor(out=d2[:], in0=delta, in1=delta,
                                        op=OP.mult)
                nc.vector.tensor_tensor(out=d2[:], in0=d2[:], in1=n_emp,
                                        op=OP.mult)
                nc.vector.tensor_tensor(out=arin[:, mparts + m:mparts + m + 1],
                                        in0=rs_[:, 1:2], in1=d2[:],
                                        op=OP.subtract)
            st = allreduce(arin[:], 2 * mparts, f"{name}_ar")
            zout = []
            for m in range(mparts):
                sc, sh = bn_params(st[:, m:m + 1],
                                   st[:, mparts + m:mparts + m + 1],
                                   g_aps[m], be_aps[m], f"{name}_p{m}")
                zo = big.tile([128, nloc], BF16, name=f"{name}_n{m}",
                              tag=zout_tags[m][1])
                nc.scalar.activation(zo[:], zraw[m][:], AF.Relu,
                                     bias=sh[:, 0:1], scale=sc[:, 0:1])
                zout.append(zo)
            return zout

        # L1: z0 [C,n] -> z1 halves; W1 [128, 2H]: lhsT for half m = w1[:, m*128...]
        z1 = mlp_layer([z0], w1, lambda k, m: m * 128, 1,
                       [("slotA", "slotB"), ("slotC", "slotD")],
                       b1, g1, be1, "l1")
        # L2: z1 (2 K-parts) -> z2 halves; W2sb [128, 512]:
        # lhsT(k,m) at col k*256 + m*128
        z2 = mlp_layer(z1, w2, lambda k, m: k * 256 + m * 128, 2,
                       [("slotA", "slotB"), ("slotC", "slotD")],
                       b2, g2, be2, "l2")

        # ---- phase 7: z3 = W3^T @ z2 + b3; u = z3 + xT; out = We^T @ u + be
        for o, w in blocks:
            ps3 = ps.tile([128, 512], F32, name="ps3", tag="mm", bufs=3)
            for k in range(2):
                nc.tensor.matmul(
                    ps3[:, :w], lhsT=w3[:, k * 128:k * 128 + 128],
                    rhs=z2[k][:, o:o + w],
                    start=(k == 0), stop=(k == 1))
            u = wk.tile([128, 512], F32R, name="u", tag="u", bufs=3)
            nc.scalar.activation(u[:, :w], ps3[:, :w], AF.Identity, bias=b3)
            nc.vector.tensor_tensor(out=u[:, :w], in0=u[:, :w],
                                    in1=xT[:, o:o + w], op=OP.add)
            ps4 = ps.tile([128, 512], F32, name="ps4", tag="mm", bufs=3)
            nc.tensor.matmul(ps4[:, :w], lhsT=we[:], rhs=u[:, :w],
                             start=True, stop=True)
            ob = wk.tile([128, 512], F32, name="ob", tag="ob", bufs=3)
            nc.scalar.activation(ob[:, :w], ps4[:, :w], AF.Identity,
                                 bias=be_enc)
            nc.sync.dma_start(out_d[:, o:o + w], ob[:, :w])

    nc.compile()
    return nc


# --------------------------------------------------------------------------
# public entry
# --------------------------------------------------------------------------

_CACHE = {}
LAST_RESULT = None


def _run(x, edge_index, bn_g, bn_b, W1, b1, g1, be1, W2, b2, g2, be2,
         W3, b3, We, be, n_nodes):
    import ml_dtypes
    bf16 = ml_dtypes.bfloat16

    src = np.asarray(edge_index[0], dtype=np.int64)
    dst = np.asarray(edge_index[1], dtype=np.int64)
    x = np.asarray(x, dtype=np.float32)

    plan = _plan(n_nodes, src, dst)
    tiles, nloc, NB = plan["tiles"], plan["nloc"], plan["NB"]

    key = (tiles, NB, tuple(plan["B"].tolist()), tuple(plan["A0"].tolist()),
           n_nodes)
    if key not in _CACHE:
        _CACHE[key] = _build(plan, n_nodes)
    nc = _CACHE[key]

    h_dim = W1.shape[1]
    # pack weights
    w2sb = np.ascontiguousarray(
        W2.reshape(2, 128, h_dim).transpose(1, 0, 2).reshape(128, 2 * h_dim)
    ).astype(bf16)
    w3sb = np.ascontiguousarray(
        W3.reshape(2, 128, 128).transpose(1, 0, 2).reshape(128, 256)
    ).astype(bf16)
    iota = np.broadcast_to(np.arange(128, dtype=np.float32),
                           (128, 128)).astype(bf16)

    halves = lambda v: [np.asarray(v[:128], np.float32),
                        np.asarray(v[128:], np.float32)]
    b1h, g1h, be1h = halves(b1), halves(g1), halves(be1)
    b2h, g2h, be2h = halves(b2), halves(g2), halves(be2)

    in_maps = []
    core_nodes = []
    for c in range(NCORES):
        sel = plan["core_of"] == c
        nodes_c = np.nonzero(sel)[0]
        core_nodes.append(nodes_c)
        xr = np.zeros((nloc, 128), np.float32)
        xr[plan["loc_of"][nodes_c]] = x[nodes_c]
        pcols = np.zeros((128, 17), np.float32)
        pcols[:, 0] = bn_g
        pcols[:, 1] = bn_b
        for i2 in range(2):
            pcols[:, 2 + i2] = b1h[i2]
            pcols[:, 4 + i2] = g1h[i2]
            pcols[:, 6 + i2] = be1h[i2]
            pcols[:, 8 + i2] = b2h[i2]
            pcols[:, 10 + i2] = g2h[i2]
            pcols[:, 12 + i2] = be2h[i2]
        pcols[:, 14] = b3
        pcols[:, 15] = be
        pcols[:, 16] = plan["n_empty"][c]
        in_maps.append({
            "xT": np.ascontiguousarray(xr.T),
            "idx16": plan["idxw"][c],
            "edst": plan["edst"][c].astype(bf16),
            "iota": iota,
            "w1": np.asarray(W1, np.float32).astype(bf16),
            "w2": w2sb,
            "w3": w3sb,
            "we": np.asarray(We, np.float32),
            "pcols": pcols,
        })

    import os
    trace = bool(os.environ.get("KTRACE"))
    res = run_bass_kernel_spmd(nc, in_maps, list(range(NCORES)), trace=trace)
    global LAST_RESULT
    LAST_RESULT = res
    out = np.empty((n_nodes, 128), np.float32)
    for c in range(NCORES):
        nodes_c = core_nodes[c]
        out[nodes_c] = res.results[c]["outT"][:, plan["loc_of"][nodes_c]].T
    return out


def kernel(x, edge_index, bn_g, bn_b, W1, b1, g1, be1, W2, b2, g2, be2,
           W3, b3, We, be):
    return _run(x, edge_index, bn_g, bn_b, W1, b1, g1, be1, W2, b2, g2, be2,
                W3, b3, We, be, n_nodes=x.shape[0])
